# revision 1
# baseline (speedup 1.0000x reference)
"""Vision-RWKV (RWKV-v4 spatial mix) encoder block on 8 Trainium2 NeuronCores.

Strategy: data-parallel over batch B=16 -> 2 batches per core, no collectives.
Layout: channel-major [c, t] on-chip (x host-transposed). The WKV recurrence
P_t = e^w * P_{t-1} + e^{k_t} v_t runs as a hardware tensor_tensor_scan along
the free (token) dim. Matmuls run in fp16 (full PE rate); scan/elementwise in
fp32. LayerNorm stats via PE ones-matmul (cross-partition reduction).

Self-contained: hardcodes B=16, T=1024, C=1024, H=W=32, 8 cores.
"""
import sys
sys.path.insert(0, "/opt/trn_rl_repo")

from contextlib import ExitStack

import numpy as np

import concourse.bacc as bacc
import concourse.tile as tile
from concourse import mybir
from concourse.bass_utils import run_bass_kernel_spmd

dt = mybir.dt
AF = mybir.ActivationFunctionType
ALU = mybir.AluOpType

B, T, C = 16, 1024, 1024
NCORES = 8
BL = B // NCORES          # batches per core
NCT = C // 128            # channel tiles (8)
TC = 512                  # token chunk
NCH = T // TC             # chunks per batch (2)
NTT = TC // 128           # token tiles per chunk (4)
EPS = 1e-5

F32, F16 = dt.float32, dt.float16

# cvec column offsets (each vector packed as [128, 8], c_tile-major columns)
MK, MV, MR, EW, EU, LW, LB = 0, 8, 16, 24, 32, 40, 48
NVEC = 56

_CACHE = {}


def _emit_dshift(nc, d16, x16, i, c, eng=None):
    """d = x - qshift(x) for channel block i, chunk c, into d16[:, i*TC : (i+1)*TC].

    x16 is the full [128, 8*1024] fp16 plane (block i at cols i*1024).
    Within a block, col index = token t (0..1023). Chunk covers t in
    [c*TC, (c+1)*TC). Group g = i//2 determines the spatial shift.
    """
    g = i // 2
    xb = c * TC              # token base of this chunk
    xo = i * 1024 + xb       # col offset of this (block, chunk) in x16
    do = i * TC              # col offset in the chunk-local d16 plane
    last = c == NCH - 1
    first = c == 0

    engine = eng if eng is not None else nc.gpsimd

    def tt_sub(lo, hi, shift):
        # d[t] = x[t] - x[t - shift] over chunk-local t in [lo, hi)
        engine.tensor_tensor(
            d16[:, do + lo:do + hi],
            x16[:, xo + lo:xo + hi],
            x16[:, xo + lo - shift:xo + hi - shift],
            ALU.subtract)

    def fix(lo, hi):
        nc.vector.tensor_copy(d16[:, do + lo:do + hi], x16[:, xo + lo:xo + hi])

    def fix_strided(col):
        # copy x -> d at chunk-local t with t % 32 == col
        sr = x16[:, xo:xo + TC].rearrange("p (a q) -> p a q", q=32)
        dr = d16[:, do:do + TC].rearrange("p (a q) -> p a q", q=32)
        nc.vector.tensor_copy(dr[:, :, col:col + 1], sr[:, :, col:col + 1])

    if g == 0:        # xx = x(h, w-1), 0 at w==0  -> shift +1
        tt_sub(1 if first else 0, TC, 1)
        fix_strided(0)                                 # t % 32 == 0
    elif g == 1:      # xx = x(h, w+1), 0 at w==31 -> shift -1
        tt_sub(0, TC - 1 if last else TC, -1)
        fix_strided(31)                                # t % 32 == 31
    elif g == 2:      # xx = x(h-1, w), 0 at h==0  -> shift +32
        tt_sub(32 if first else 0, TC, 32)
        if first:
            fix(0, 32)
    else:             # xx = x(h+1, w), 0 at h==31 -> shift -32
        tt_sub(0, TC - 32 if last else TC, -32)
        if last:
            fix(TC - 32, TC)


def build():
    nc = bacc.Bacc("TRN2", target_bir_lowering=False, debug=False)

    x_d = nc.dram_tensor("x", [BL, C, T], F16, kind="ExternalInput")
    wk_d = nc.dram_tensor("wk", [NCT, C, 128], F16, kind="ExternalInput")
    wv_d = nc.dram_tensor("wv", [NCT, C, 128], F16, kind="ExternalInput")
    wr_d = nc.dram_tensor("wr", [C, C], F16, kind="ExternalInput")
    wo_d = nc.dram_tensor("wo", [C, C], F16, kind="ExternalInput")
    cvec_d = nc.dram_tensor("cvec", [128, NVEC], F32, kind="ExternalInput")
    out_d = nc.dram_tensor("out", [BL, T, C], F32, kind="ExternalOutput")

    with tile.TileContext(nc) as tc, ExitStack() as ctx:
        # ---- pools ----
        cons = ctx.enter_context(tc.tile_pool(name="cons", bufs=1))
        wp = ctx.enter_context(tc.tile_pool(name="wp", bufs=1))
        xpl = ctx.enter_context(tc.tile_pool(name="xpl", bufs=2))
        dpl = ctx.enter_context(tc.tile_pool(name="dpl", bufs=1))
        mxp = ctx.enter_context(tc.tile_pool(name="mxp", bufs=4))
        ekp = ctx.enter_context(tc.tile_pool(name="ekp", bufs=2))
        pqp = ctx.enter_context(tc.tile_pool(name="pqp", bufs=2))
        ndp = ctx.enter_context(tc.tile_pool(name="ndp", bufs=1))
        ypl = ctx.enter_context(tc.tile_pool(name="ypl", bufs=1))
        ysq = ctx.enter_context(tc.tile_pool(name="ysq", bufs=2))
        stp = ctx.enter_context(tc.tile_pool(name="stp", bufs=2))
        gtp = ctx.enter_context(tc.tile_pool(name="gtp", bufs=6))
        srp = ctx.enter_context(tc.tile_pool(name="srp", bufs=1))
        syp = ctx.enter_context(tc.tile_pool(name="syp", bufs=1))
        osg = ctx.enter_context(tc.tile_pool(name="osg", bufs=2))
        carp = ctx.enter_context(tc.tile_pool(name="carp", bufs=1))

        pp = ctx.enter_context(tc.tile_pool(name="pp", bufs=4, space="PSUM"))
        sps = ctx.enter_context(tc.tile_pool(name="sps", bufs=1, space="PSUM"))
        opl = ctx.enter_context(tc.tile_pool(name="opl", bufs=2, space="PSUM"))

        # ---- constants ----
        cvec = cons.tile([128, NVEC], F32)
        nc.sync.dma_start(cvec[:], cvec_d.ap()[:])
        ones16 = cons.tile([128, 1], F16)
        nc.vector.memset(ones16[:], 1.0 / C)
        eps_t = cons.tile([1, 1], F32)
        nc.vector.memset(eps_t[:], EPS)

        # PE warmup during the initial x/weight DMA wait: keeps the HAM
        # clock-gate and p-state warm so the first real matmuls run at 2.4GHz
        wu = cons.tile([128, 128], F16)
        nc.vector.memset(wu[:], 0.5)
        wu_ps = pp.tile([128, 128], F32, tag="proj")
        for _ in range(60):
            nc.tensor.matmul(wu_ps[:], wu[:], wu[:], start=True, stop=True)

        wk = wp.tile([128, NCT * C], F16, tag="wk")
        wv = wp.tile([128, NCT * C], F16, tag="wv")
        wr = wp.tile([128, NCT * C], F16, tag="wr")
        wo = wp.tile([128, NCT * C], F16, tag="wo")
        def dma_w(w_sb, w_d):
            nc.scalar.dma_start(
                w_sb[:].rearrange("p (i d) -> p i d", d=C),
                w_d.ap().rearrange("(i p) d -> p i d", p=128))

        def dma_x(b):
            x16 = xpl.tile([128, NCT * 1024], F16, tag="x16")
            half = NCT * 1024 // 2
            xsrc = x_d.ap()[b].rearrange("(i p) t -> p i t", p=128)
            for h in range(2):
                nc.sync.dma_start(
                    x16[:, h * half:(h + 1) * half]
                    .rearrange("p (i t) -> p i t", t=1024),
                    xsrc[:, h * 4:(h + 1) * 4, :])
            return x16

        pending_out = []

        def emit_out(ob, och, sry):
            for m in range(NTT):
                og = osg.tile([128, C], F32, tag="ostg")
                for dh in range(2):
                    o_ps = opl.tile([128, TC], F32, tag="oproj")
                    for i in range(NCT):
                        nc.tensor.matmul(
                            o_ps[:],
                            sry[:, i * TC + m * 128:i * TC + (m + 1) * 128],
                            wo[:, i * C + dh * TC:i * C + (dh + 1) * TC],
                            start=(i == 0), stop=(i == NCT - 1))
                    if ob == BL - 1 and och == NCH - 1:
                        nc.vector.tensor_copy(
                            og[:, dh * TC:(dh + 1) * TC], o_ps[:])
                        eng = nc.sync if dh == 0 else nc.scalar
                        trow = (och * NTT + m) * 128
                        eng.dma_start(
                            out_d.ap()[ob, trow:trow + 128,
                                       dh * TC:(dh + 1) * TC],
                            og[:, dh * TC:(dh + 1) * TC])
                    else:
                        nc.scalar.copy(og[:, dh * TC:(dh + 1) * TC], o_ps[:])
                if not (ob == BL - 1 and och == NCH - 1):
                    trow = (och * NTT + m) * 128
                    nc.sync.dma_start(
                        out_d.ap()[ob, trow:trow + 128, :], og[:])

        x16_first = dma_x(0)

        def dma_wj(eng, w_sb, w_d, j):
            eng.dma_start(
                w_sb[:, j * C:(j + 1) * C].rearrange("p (i d) -> p i d", d=128),
                w_d.ap()[j].rearrange("(i p) d -> p i d", p=128))
        for j in range(NCT):
            dma_wj(nc.scalar, wk, wk_d, j)
            dma_wj(nc.sync, wv, wv_d, j)

        for b in range(BL):
            # ---- ingest x (host-transposed, host-cast fp16): one DMA ----
            x16 = x16_first if b == 0 else _CACHE.pop("x16_next")

            if b == 0:
                dma_w(wr, wr_d)
                dma_w(wo, wo_d)

            carryP = carp.tile([128, NCT], F32, tag="cp")
            carryQ = carp.tile([128, NCT], F32, tag="cq")

            def emit_d(ch, fast=False):
                d16 = dpl.tile([128, NCT * TC], F16, tag="d16")
                for i in range(NCT):
                    _emit_dshift(nc, d16, x16, i, ch, eng=nc.vector)
                return d16

            def emit_mix_block(ch, d16, mx, off, i):
                s = slice(i * TC, (i + 1) * TC)
                xs = slice(i * 1024 + ch * TC, i * 1024 + ch * TC + TC)
                nc.vector.scalar_tensor_tensor(
                    mx[:, s], d16[:, s], cvec[:, off + i:off + i + 1],
                    x16[:, xs], ALU.mult, ALU.add)

            def emit_mix(ch, d16, off):
                mx = mxp.tile([128, NCT * TC], F16, tag="mx")
                for i in range(NCT):
                    emit_mix_block(ch, d16, mx, off, i)
                return mx

            d_cur = emit_d(0, fast=True)
            xr_head = mxp.tile([128, NCT * TC], F16, tag="mx")
            mix_next = (emit_mix(0, d_cur, MK), emit_mix(0, d_cur, MV),
                        ("lazy", xr_head, d_cur))
            for ch in range(NCH):
                xk, xv, xr = mix_next
                xr_lazy = None
                if isinstance(xr, tuple):
                    xr_lazy, xr, d_lazy = xr
                y16 = ypl.tile([128, NCT * TC], F16, tag="y16")
                if ch + 1 < NCH:
                    d_nxt = emit_d(ch + 1)
                    xk_nxt = mxp.tile([128, NCT * TC], F16, tag="mx")

                for j in range(NCT):
                    # ---- k, v projections for output-channel block j ----
                    k_ps = pp.tile([128, TC], F32, tag="proj")
                    v_ps = pp.tile([128, TC], F32, tag="proj")
                    for w_sb, xm, ps in [(wk, xk, k_ps), (wv, xv, v_ps)]:
                        for i in range(NCT):
                            base = (j * NCT + i) * 128
                            nc.tensor.matmul(
                                ps[:],
                                w_sb[:, base:base + 128],
                                xm[:, i * TC:(i + 1) * TC],
                                start=(i == 0), stop=(i == NCT - 1))

                    ek = ekp.tile([128, TC], F32, tag="ek")
                    nc.scalar.activation(ek[:], k_ps[:], AF.Exp)
                    ekv = ekp.tile([128, TC], F32, tag="ekv")
                    nc.vector.tensor_mul(ekv[:], ek[:], v_ps[:])

                    # ---- WKV scan ----
                    pbuf = pqp.tile([128, TC + 1], F32, tag="pbuf")
                    qbuf = pqp.tile([128, TC + 1], F32, tag="qbuf")
                    if ch == 0:
                        nc.vector.memset(pbuf[:, 0:1], 0.0)
                        nc.vector.memset(qbuf[:, 0:1], 0.0)
                    else:
                        nc.scalar.copy(pbuf[:, 0:1], carryP[:, j:j + 1])
                        nc.scalar.copy(qbuf[:, 0:1], carryQ[:, j:j + 1])
                    ewb = cvec[:, EW + j:EW + j + 1].broadcast_to([128, TC])
                    nc.vector.tensor_tensor_scan(
                        pbuf[:, 1:TC + 1], ewb, ekv[:], pbuf[:, 0:1],
                        ALU.mult, ALU.add)
                    nc.vector.tensor_tensor_scan(
                        qbuf[:, 1:TC + 1], ewb, ek[:], qbuf[:, 0:1],
                        ALU.mult, ALU.add)
                    if ch != NCH - 1:
                        nc.scalar.copy(carryP[:, j:j + 1], pbuf[:, TC:TC + 1])
                        nc.scalar.copy(carryQ[:, j:j + 1], qbuf[:, TC:TC + 1])

                    # ---- y = (P_{t-1} + e^{u+k} v) / (Q_{t-1} + e^{u+k}) ----
                    num = ndp.tile([128, TC], F32, tag="num")
                    den = ndp.tile([128, TC], F32, tag="den")
                    eu_c = cvec[:, EU + j:EU + j + 1]
                    nc.vector.scalar_tensor_tensor(
                        num[:], ekv[:], eu_c, pbuf[:, 0:TC], ALU.mult, ALU.add)
                    nc.vector.scalar_tensor_tensor(
                        den[:], ek[:], eu_c, qbuf[:, 0:TC], ALU.mult, ALU.add)
                    nc.vector.reciprocal_approx_fast(den[:], den[:])
                    yb = y16[:, j * TC:(j + 1) * TC]
                    nc.gpsimd.tensor_mul(yb, num[:], den[:])

                    if xr_lazy:
                        emit_mix_block(ch, d_lazy, xr, MR, j)
                    if ch + 1 < NCH:
                        emit_mix_block(ch + 1, d_nxt, xk_nxt, MK, j)

                while pending_out:
                    emit_out(*pending_out.pop(0))

                # ---- LN stats: lhsT ones = 1/C so psum rows are mu, E[y2]
                st_ps = sps.tile([1, 2 * TC], F32, tag="stat")
                for j in range(NCT):
                    yb = y16[:, j * TC:(j + 1) * TC]
                    ys = ysq.tile([128, TC], F16, tag="ysq")
                    nc.scalar.square(ys[:], yb)
                    nc.tensor.matmul(st_ps[:, 0:TC], ones16[:], yb,
                                     start=(j == 0), stop=(j == NCT - 1))
                    nc.tensor.matmul(st_ps[:, TC:2 * TC], ones16[:], ys[:],
                                     start=(j == 0), stop=(j == NCT - 1))

                # ---- r projections + sigmoid (independent of LN) ----
                sr16 = srp.tile([128, NCT * TC], F16, tag="sr")
                for j in range(NCT):
                    r_ps = pp.tile([128, TC], F32, tag="proj")
                    for i in range(NCT):
                        nc.tensor.matmul(
                            r_ps[:],
                            wr[:, i * C + j * 128:i * C + (j + 1) * 128],
                            xr[:, i * TC:(i + 1) * TC],
                            start=(i == 0), stop=(i == NCT - 1))
                    nc.scalar.activation(sr16[:, j * TC:(j + 1) * TC],
                                         r_ps[:], AF.Sigmoid)

                if ch == NCH - 1 and b + 1 < BL:
                    _CACHE["x16_next"] = dma_x(b + 1)

                # ---- next chunk's xv/xr fill DVE while PE runs r/out ----
                if ch + 1 < NCH:
                    mix_next = (xk_nxt, emit_mix(ch + 1, d_nxt, MV),
                                emit_mix(ch + 1, d_nxt, MR))

                # ---- LN stats post (rows on partition 0) ----
                mu16 = stp.tile([1, TC], F16, tag="mu16")
                nc.scalar.copy(mu16[:], st_ps[:, 0:TC])
                ms_t = stp.tile([1, TC], F32, tag="strow")
                nc.scalar.square(ms_t[:], st_ps[:, 0:TC])
                var_t = stp.tile([1, TC], F32, tag="strow")
                nc.vector.tensor_sub(var_t[:], st_ps[:, TC:2 * TC], ms_t[:])
                sd_t = stp.tile([1, TC], F32, tag="strow")
                nc.scalar.activation(sd_t[:], var_t[:], AF.Sqrt, bias=eps_t[:])
                nc.vector.reciprocal_approx_fast(sd_t[:], sd_t[:])
                rs16 = stp.tile([1, TC], F16, tag="rs16")
                nc.vector.tensor_scalar(rs16[:], sd_t[:], 1.0, None, ALU.mult)
                rsb = stp.tile([128, TC], F16, tag="bcast")
                nc.gpsimd.partition_broadcast(rsb[:], rs16[:])
                mub = stp.tile([128, TC], F16, tag="bcast")
                nc.gpsimd.partition_broadcast(mub[:], mu16[:])

                # ---- gate: sry = sr * ((y - mu)*rstd*lnw + lnb) ----
                sry = syp.tile([128, NCT * TC], F16, tag="sry")
                last_chunk = (b == BL - 1 and ch == NCH - 1)
                for j in range(NCT):
                    srb = sr16[:, j * TC:(j + 1) * TC]
                    ya = gtp.tile([128, TC], F16, tag="gt")
                    nc.vector.tensor_sub(ya[:], y16[:, j * TC:(j + 1) * TC],
                                         mub[:])
                    ybt = gtp.tile([128, TC], F16, tag="gt")
                    eng = nc.vector if last_chunk else nc.gpsimd
                    eng.tensor_mul(ybt[:], ya[:], rsb[:])
                    yct = gtp.tile([128, TC], F16, tag="gt")
                    nc.scalar.activation(
                        yct[:], ybt[:], AF.Identity,
                        bias=cvec[:, LB + j:LB + j + 1],
                        scale=cvec[:, LW + j:LW + j + 1])
                    nc.vector.tensor_mul(sry[:, j * TC:(j + 1) * TC],
                                         yct[:], srb)

                pending_out.append((b, ch, sry))
                if last_chunk:
                    while pending_out:
                        emit_out(*pending_out.pop(0))

    nc.compile()
    return nc


def _pack(v):
    return np.ascontiguousarray(v.reshape(NCT, 128).T.astype(np.float32))


def kernel(x, Wk, Wv, Wr, Wo, ln_w, ln_b, spatial_decay, spatial_first,
           mix_k, mix_v, mix_r, H, W):
    x = np.asarray(x, dtype=np.float32)
    assert int(H) == 32 and int(W) == 32 and x.shape == (B, T, C)

    if "nc" not in _CACHE:
        _CACHE["nc"] = build()
    nc = _CACHE["nc"]

    w_eff = -np.exp(np.asarray(spatial_decay, np.float64) / T)
    u_eff = np.asarray(spatial_first, np.float64) / T
    cvec = np.concatenate([
        _pack(np.asarray(mix_k, np.float32) - 1.0),
        _pack(np.asarray(mix_v, np.float32) - 1.0),
        _pack(np.asarray(mix_r, np.float32) - 1.0),
        _pack(np.exp(w_eff).astype(np.float32)),
        _pack(np.exp(u_eff).astype(np.float32)),
        _pack(np.asarray(ln_w, np.float32)),
        _pack(np.asarray(ln_b, np.float32)),
    ], axis=1)
    def _jmajor(W):
        wT = np.asarray(W, np.float32).T.astype(np.float16)   # [C, D]
        return np.ascontiguousarray(
            wT.reshape(C, NCT, 128).transpose(1, 0, 2))       # [j, c, dd]
    wk16 = _jmajor(Wk)
    wv16 = _jmajor(Wv)
    wr16 = np.ascontiguousarray(np.asarray(Wr, np.float32).T).astype(np.float16)
    wo16 = np.ascontiguousarray(np.asarray(Wo, np.float32).T).astype(np.float16)
    x_t = np.ascontiguousarray(x.transpose(0, 2, 1)).astype(np.float16)

    in_maps = []
    for c in range(NCORES):
        in_maps.append({
            "x": x_t[c * BL:(c + 1) * BL],
            "wk": wk16, "wv": wv16, "wr": wr16, "wo": wo16,
            "cvec": cvec,
        })
    last_err = None
    for _attempt in range(3):
        try:
            res = run_bass_kernel_spmd(nc, in_maps,
                                       core_ids=list(range(NCORES)))
            break
        except Exception as e:  # transient device wedge: retry
            last_err = e
            import time as _time
            _time.sleep(2.0)
    else:
        raise last_err
    out = np.concatenate([res.results[c]["out"] for c in range(NCORES)], axis=0)
    return out.astype(np.float32)



# revision 7
# speedup vs baseline: 1.0076x; 1.0076x over previous
"""Vision-RWKV (RWKV-v4 spatial mix) encoder block on 8 Trainium2 NeuronCores.

Strategy: data-parallel over batch B=16 -> 2 batches per core, no collectives.
Layout: channel-major [c, t] on-chip (x host-transposed). The WKV recurrence
P_t = e^w * P_{t-1} + e^{k_t} v_t runs as a hardware tensor_tensor_scan along
the free (token) dim. K/V/O matmuls run in fp16 (full PE rate); the R
projection runs in fp8e4 DoubleRow mode (2 contraction tiles per instruction
at 0.5 cyc/row) with weights host-prescaled by 256 and descaled for free via
the activation scale; the sigmoid gate is computed as 0.5*tanh(r/2)+0.5 so
Exp/Tanh/Square/Identity stay within one activation table set. e^u ~= 1
(|u|<=5e-4) is dropped. P-scan/num in fp16 (DVE 2x mode), q/den/recip fp32.
LayerNorm stats via PE ones-matmul (cross-partition reduction).

Self-contained: hardcodes B=16, T=1024, C=1024, H=W=32, 8 cores.
"""
import sys
sys.path.insert(0, "/opt/trn_rl_repo")

from contextlib import ExitStack

import numpy as np
import ml_dtypes

import concourse.bacc as bacc
import concourse.tile as tile
from concourse import mybir
from concourse.bass_utils import run_bass_kernel_spmd

dt = mybir.dt
AF = mybir.ActivationFunctionType
ALU = mybir.AluOpType
DRM = mybir.MatmulPerfMode.DoubleRow

B, T, C = 16, 1024, 1024
NCORES = 8
BL = B // NCORES          # batches per core
NCT = C // 128            # channel tiles (8)
TC = 512                  # token chunk
NCH = T // TC             # chunks per batch (2)
NTT = TC // 128           # token tiles per chunk (4)
EPS = 1e-5
SR_W = 256.0              # host prescale on R weights (fp8 subnormal avoidance)

F32, F16, F8 = dt.float32, dt.float16, dt.float8e4
E4 = ml_dtypes.float8_e4m3

# cvec column offsets (each vector packed as [128, 8], c_tile-major columns)
MK, MV, MR, EW, EU, LW, LB = 0, 8, 16, 24, 32, 40, 48
NVEC = 56

_CACHE = {}


def _emit_dshift(nc, d16, x16, i, c, eng=None):
    """d = x - qshift(x) for channel block i, chunk c, into d16[:, i*TC : (i+1)*TC].

    x16 is the full [128, 8*1024] fp16 plane (block i at cols i*1024).
    Within a block, col index = token t (0..1023). Chunk covers t in
    [c*TC, (c+1)*TC). Group g = i//2 determines the spatial shift.
    """
    g = i // 2
    xb = c * TC              # token base of this chunk
    xo = i * 1024 + xb       # col offset of this (block, chunk) in x16
    do = i * TC              # col offset in the chunk-local d16 plane
    last = c == NCH - 1
    first = c == 0

    engine = eng if eng is not None else nc.gpsimd

    def tt_sub(lo, hi, shift):
        # d[t] = x[t] - x[t - shift] over chunk-local t in [lo, hi)
        engine.tensor_tensor(
            d16[:, do + lo:do + hi],
            x16[:, xo + lo:xo + hi],
            x16[:, xo + lo - shift:xo + hi - shift],
            ALU.subtract)

    def fix(lo, hi):
        nc.vector.tensor_copy(d16[:, do + lo:do + hi], x16[:, xo + lo:xo + hi])

    def fix_strided(col):
        # copy x -> d at chunk-local t with t % 32 == col
        sr = x16[:, xo:xo + TC].rearrange("p (a q) -> p a q", q=32)
        dr = d16[:, do:do + TC].rearrange("p (a q) -> p a q", q=32)
        nc.vector.tensor_copy(dr[:, :, col:col + 1], sr[:, :, col:col + 1])

    if g == 0:        # xx = x(h, w-1), 0 at w==0  -> shift +1
        tt_sub(1 if first else 0, TC, 1)
        fix_strided(0)                                 # t % 32 == 0
    elif g == 1:      # xx = x(h, w+1), 0 at w==31 -> shift -1
        tt_sub(0, TC - 1 if last else TC, -1)
        fix_strided(31)                                # t % 32 == 31
    elif g == 2:      # xx = x(h-1, w), 0 at h==0  -> shift +32
        tt_sub(32 if first else 0, TC, 32)
        if first:
            fix(0, 32)
    else:             # xx = x(h+1, w), 0 at h==31 -> shift -32
        tt_sub(0, TC - 32 if last else TC, -32)
        if last:
            fix(TC - 32, TC)


def build():
    nc = bacc.Bacc("TRN2", target_bir_lowering=False, debug=False)

    x_d = nc.dram_tensor("x", [BL, C, T], F16, kind="ExternalInput")
    wk_d = nc.dram_tensor("wk", [NCT, C, 128], F16, kind="ExternalInput")
    wv_d = nc.dram_tensor("wv", [NCT, C, 128], F16, kind="ExternalInput")
    wr8_d = nc.dram_tensor("wr8", [128, 4 * 2 * NCT * 128], F8,
                           kind="ExternalInput")
    wo_d = nc.dram_tensor("wo", [C, C], F16, kind="ExternalInput")
    cvec_d = nc.dram_tensor("cvec", [128, NVEC], F32, kind="ExternalInput")
    out_d = nc.dram_tensor("out", [BL, T, C], F32, kind="ExternalOutput")

    with tile.TileContext(nc) as tc, ExitStack() as ctx:
        # ---- pools ----
        cons = ctx.enter_context(tc.tile_pool(name="cons", bufs=1))
        wp = ctx.enter_context(tc.tile_pool(name="wp", bufs=1))
        xpl = ctx.enter_context(tc.tile_pool(name="xpl", bufs=2))
        dpl = ctx.enter_context(tc.tile_pool(name="dpl", bufs=1))
        mxp = ctx.enter_context(tc.tile_pool(name="mxp", bufs=4))
        ekp = ctx.enter_context(tc.tile_pool(name="ekp", bufs=2))
        pqp = ctx.enter_context(tc.tile_pool(name="pqp", bufs=2))
        ndp = ctx.enter_context(tc.tile_pool(name="ndp", bufs=1))
        ypl = ctx.enter_context(tc.tile_pool(name="ypl", bufs=1))
        ysq = ctx.enter_context(tc.tile_pool(name="ysq", bufs=2))
        stp = ctx.enter_context(tc.tile_pool(name="stp", bufs=2))
        gtp = ctx.enter_context(tc.tile_pool(name="gtp", bufs=6))
        srp = ctx.enter_context(tc.tile_pool(name="srp", bufs=1))
        syp = ctx.enter_context(tc.tile_pool(name="syp", bufs=1))
        osg = ctx.enter_context(tc.tile_pool(name="osg", bufs=1))
        carp = ctx.enter_context(tc.tile_pool(name="carp", bufs=1))

        pp = ctx.enter_context(tc.tile_pool(name="pp", bufs=4, space="PSUM"))
        sps = ctx.enter_context(tc.tile_pool(name="sps", bufs=1, space="PSUM"))
        opl = ctx.enter_context(tc.tile_pool(name="opl", bufs=2, space="PSUM"))

        # ---- constants ----
        cvec = cons.tile([128, NVEC], F32)
        nc.sync.dma_start(cvec[:], cvec_d.ap()[:])
        ones16 = cons.tile([128, 1], F16)
        nc.vector.memset(ones16[:], 1.0 / C)
        eps_t = cons.tile([1, 1], F32)
        nc.vector.memset(eps_t[:], EPS)
        s2r = cons.tile([128, 1], F32)
        nc.vector.memset(s2r[:], 1.0 / (2.0 * SR_W))   # tanh(r/2) descale

        # PE warmup during the initial x/weight DMA wait: keeps the HAM
        # clock-gate and p-state warm so the first real matmuls run at 2.4GHz
        wu = cons.tile([128, 128], F16)
        nc.vector.memset(wu[:], 0.5)
        wu_ps = pp.tile([128, 128], F32, tag="proj")
        for _ in range(60):
            nc.tensor.matmul(wu_ps[:], wu[:], wu[:], start=True, stop=True)

        wk = wp.tile([128, NCT * C], F16, tag="wk")
        wv = wp.tile([128, NCT * C], F16, tag="wv")
        wr8 = wp.tile([128, 4, 2, NCT, 128], F8, tag="wr8")
        wo = wp.tile([128, NCT * C], F16, tag="wo")

        # ew16[:, j*TC:(j+1)*TC] = exp(w_eff) for channel block j, along free
        ew16 = cons.tile([128, NCT * TC], F16)
        nc.vector.memset(ew16[:], 1.0)
        for j in range(NCT):
            nc.vector.tensor_scalar(
                ew16[:, j * TC:(j + 1) * TC], ew16[:, j * TC:(j + 1) * TC],
                cvec[:, EW + j:EW + j + 1], None, ALU.mult)

        def dma_w(w_sb, w_d):
            nc.scalar.dma_start(
                w_sb[:].rearrange("p (i d) -> p i d", d=C),
                w_d.ap().rearrange("(i p) d -> p i d", p=128))

        def dma_x(b):
            x16 = xpl.tile([128, NCT * 1024], F16, tag="x16")
            half = NCT * 1024 // 2
            xsrc = x_d.ap()[b].rearrange("(i p) t -> p i t", p=128)
            for h in range(2):
                nc.sync.dma_start(
                    x16[:, h * half:(h + 1) * half]
                    .rearrange("p (i t) -> p i t", t=1024),
                    xsrc[:, h * 4:(h + 1) * 4, :])
            return x16

        pending_out = []

        def emit_out(ob, och, sry):
            for m in range(NTT):
                og = osg.tile([128, C], F32, tag="ostg")
                for dh in range(2):
                    o_ps = opl.tile([128, TC], F32, tag="oproj")
                    for i in range(NCT):
                        nc.tensor.matmul(
                            o_ps[:],
                            sry[:, i * TC + m * 128:i * TC + (m + 1) * 128],
                            wo[:, i * C + dh * TC:i * C + (dh + 1) * TC],
                            start=(i == 0), stop=(i == NCT - 1))
                    if ob == BL - 1 and och == NCH - 1:
                        nc.vector.tensor_copy(
                            og[:, dh * TC:(dh + 1) * TC], o_ps[:])
                        eng = nc.sync if dh == 0 else nc.scalar
                        trow = (och * NTT + m) * 128
                        eng.dma_start(
                            out_d.ap()[ob, trow:trow + 128,
                                       dh * TC:(dh + 1) * TC],
                            og[:, dh * TC:(dh + 1) * TC])
                    else:
                        nc.scalar.copy(og[:, dh * TC:(dh + 1) * TC], o_ps[:])
                if not (ob == BL - 1 and och == NCH - 1):
                    trow = (och * NTT + m) * 128
                    nc.sync.dma_start(
                        out_d.ap()[ob, trow:trow + 128, :], og[:])

        x16_first = dma_x(0)

        def dma_wj(eng, w_sb, w_d, j):
            eng.dma_start(
                w_sb[:, j * C:(j + 1) * C].rearrange("p (i d) -> p i d", d=128),
                w_d.ap()[j].rearrange("(i p) d -> p i d", p=128))
        for j in range(NCT):
            dma_wj(nc.scalar, wk, wk_d, j)
            dma_wj(nc.sync, wv, wv_d, j)

        for b in range(BL):
            # ---- ingest x (host-transposed, host-cast fp16): one DMA ----
            x16 = x16_first if b == 0 else _CACHE.pop("x16_next")

            if b == 0:
                nc.scalar.dma_start(
                    wr8[:].rearrange("p a b c d -> p (a b c d)"),
                    wr8_d.ap()[:])
                dma_w(wo, wo_d)

            carryP = carp.tile([128, NCT], F16, tag="cp")
            carryQ = carp.tile([128, NCT], F32, tag="cq")

            def emit_d(ch):
                d16 = dpl.tile([128, NCT * TC], F16, tag="d16")
                for i in range(NCT):
                    _emit_dshift(nc, d16, x16, i, ch, eng=nc.vector)
                return d16

            def emit_mix_block(ch, d16, mx, off, i):
                s = slice(i * TC, (i + 1) * TC)
                xs = slice(i * 1024 + ch * TC, i * 1024 + ch * TC + TC)
                nc.vector.scalar_tensor_tensor(
                    mx[:, s], d16[:, s], cvec[:, off + i:off + i + 1],
                    x16[:, xs], ALU.mult, ALU.add)

            def emit_mix(ch, d16, off, fp8=False):
                mx = mxp.tile([128, NCT * TC], F8 if fp8 else F16,
                              tag="mx8" if fp8 else "mx",
                              bufs=2 if fp8 else None)
                for i in range(NCT):
                    emit_mix_block(ch, d16, mx, off, i)
                return mx

            d_cur = emit_d(0)
            xr_head = mxp.tile([128, NCT * TC], F8, tag="mx8", bufs=2)
            mix_next = (emit_mix(0, d_cur, MK), emit_mix(0, d_cur, MV),
                        ("lazy", xr_head, d_cur))
            for ch in range(NCH):
                xk, xv, xr = mix_next
                xr_lazy = None
                if isinstance(xr, tuple):
                    xr_lazy, xr, d_lazy = xr
                y16 = ypl.tile([128, NCT * TC], F16, tag="y16")
                if ch + 1 < NCH:
                    d_nxt = emit_d(ch + 1)
                    xk_nxt = mxp.tile([128, NCT * TC], F16, tag="mx")

                for j in range(NCT):
                    # ---- k, v projections for output-channel block j ----
                    k_ps = pp.tile([128, TC], F32, tag="proj")
                    v_ps = pp.tile([128, TC], F32, tag="proj")
                    for w_sb, xm, ps in [(wk, xk, k_ps), (wv, xv, v_ps)]:
                        for i in range(NCT):
                            base = (j * NCT + i) * 128
                            nc.tensor.matmul(
                                ps[:],
                                w_sb[:, base:base + 128],
                                xm[:, i * TC:(i + 1) * TC],
                                start=(i == 0), stop=(i == NCT - 1))

                    ek = ekp.tile([128, TC], F32, tag="ek")
                    nc.scalar.activation(ek[:], k_ps[:], AF.Exp)
                    ekv = ekp.tile([128, TC], F16, tag="ekv")
                    nc.vector.tensor_mul(ekv[:], ek[:], v_ps[:])

                    # ---- WKV scan (p fp16, q fp32; e^u ~= 1 dropped) ----
                    pbuf = pqp.tile([128, TC + 1], F16, tag="pbuf")
                    qbuf = pqp.tile([128, TC + 1], F32, tag="qbuf")
                    if ch == 0:
                        nc.vector.memset(pbuf[:, 0:1], 0.0)
                        nc.vector.memset(qbuf[:, 0:1], 0.0)
                    else:
                        nc.scalar.copy(pbuf[:, 0:1], carryP[:, j:j + 1])
                        nc.scalar.copy(qbuf[:, 0:1], carryQ[:, j:j + 1])
                    ewj = ew16[:, j * TC:(j + 1) * TC]
                    nc.vector.tensor_tensor_scan(
                        pbuf[:, 1:TC + 1], ewj, ekv[:], pbuf[:, 0:1],
                        ALU.mult, ALU.add)
                    nc.vector.tensor_tensor_scan(
                        qbuf[:, 1:TC + 1], ewj, ek[:], qbuf[:, 0:1],
                        ALU.mult, ALU.add)
                    if ch != NCH - 1:
                        nc.scalar.copy(carryP[:, j:j + 1], pbuf[:, TC:TC + 1])
                        nc.scalar.copy(carryQ[:, j:j + 1], qbuf[:, TC:TC + 1])

                    # ---- y = (P_{t-1} + e^k v) / (Q_{t-1} + e^k) ----
                    num = ndp.tile([128, TC], F16, tag="num")
                    den = ndp.tile([128, TC], F32, tag="den")
                    nc.vector.tensor_add(num[:], pbuf[:, 0:TC], ekv[:])
                    nc.vector.tensor_add(den[:], qbuf[:, 0:TC], ek[:])
                    nc.vector.reciprocal_approx_fast(den[:], den[:])
                    yb = y16[:, j * TC:(j + 1) * TC]
                    nc.gpsimd.tensor_mul(yb, num[:], den[:])

                    if xr_lazy:
                        emit_mix_block(ch, d_lazy, xr, MR, j)
                    if ch + 1 < NCH:
                        emit_mix_block(ch + 1, d_nxt, xk_nxt, MK, j)

                while pending_out:
                    emit_out(*pending_out.pop(0))

                # ---- LN stats: lhsT ones = 1/C so psum rows are mu, E[y2]
                st_ps = sps.tile([1, 2 * TC], F32, tag="stat")
                for j in range(NCT):
                    yb = y16[:, j * TC:(j + 1) * TC]
                    ys = ysq.tile([128, TC], F16, tag="ysq")
                    nc.gpsimd.tensor_mul(ys[:], yb, yb)
                    nc.tensor.matmul(st_ps[:, 0:TC], ones16[:], yb,
                                     start=(j == 0), stop=(j == NCT - 1))
                    nc.tensor.matmul(st_ps[:, TC:2 * TC], ones16[:], ys[:],
                                     start=(j == 0), stop=(j == NCT - 1))

                # ---- r projection (fp8 DoubleRow) + tanh-gate ----
                xr8 = xr.rearrange("p (i t) -> p i t", t=TC)
                sr16 = srp.tile([128, NCT * TC], F16, tag="sr")
                for j in range(NCT):
                    r_ps = pp.tile([128, TC], F32, tag="proj")
                    for i2 in range(4):
                        nc.tensor.matmul(
                            r_ps[:],
                            wr8[:, i2, :, j, :],
                            xr8[:, 2 * i2:2 * i2 + 2, :],
                            start=(i2 == 0), stop=(i2 == 3),
                            perf_mode=DRM)
                    th = gtp.tile([128, TC], F16, tag="th", bufs=2)
                    nc.scalar.activation(th[:], r_ps[:], AF.Tanh, scale=s2r[:])
                    nc.vector.tensor_scalar(sr16[:, j * TC:(j + 1) * TC],
                                            th[:], 0.5, 0.5,
                                            ALU.mult, ALU.add)

                if ch == NCH - 1 and b + 1 < BL:
                    _CACHE["x16_next"] = dma_x(b + 1)

                # ---- next chunk's xv/xr fill DVE while PE runs r/out ----
                if ch + 1 < NCH:
                    mix_next = (xk_nxt, emit_mix(ch + 1, d_nxt, MV),
                                emit_mix(ch + 1, d_nxt, MR, fp8=True))

                # ---- LN stats post (rows on partition 0) ----
                mu16 = stp.tile([1, TC], F16, tag="mu16")
                nc.scalar.copy(mu16[:], st_ps[:, 0:TC])
                ms_t = stp.tile([1, TC], F32, tag="strow")
                nc.scalar.square(ms_t[:], st_ps[:, 0:TC])
                var_t = stp.tile([1, TC], F32, tag="strow")
                nc.vector.tensor_sub(var_t[:], st_ps[:, TC:2 * TC], ms_t[:])
                sd_t = stp.tile([1, TC], F32, tag="strow")
                nc.scalar.activation(sd_t[:], var_t[:], AF.Sqrt, bias=eps_t[:])
                nc.vector.reciprocal_approx_fast(sd_t[:], sd_t[:])
                rs16 = stp.tile([1, TC], F16, tag="rs16")
                nc.vector.tensor_scalar(rs16[:], sd_t[:], 1.0, None, ALU.mult)
                rsb = stp.tile([128, TC], F16, tag="bcast")
                nc.gpsimd.partition_broadcast(rsb[:], rs16[:])
                mub = stp.tile([128, TC], F16, tag="bcast")
                nc.gpsimd.partition_broadcast(mub[:], mu16[:])

                # ---- gate: sry = sr * ((y - mu)*rstd*lnw + lnb) ----
                sry = syp.tile([128, NCT * TC], F16, tag="sry")
                last_chunk = (b == BL - 1 and ch == NCH - 1)
                for j in range(NCT):
                    srb = sr16[:, j * TC:(j + 1) * TC]
                    ya = gtp.tile([128, TC], F16, tag="gt")
                    nc.gpsimd.tensor_sub(ya[:], y16[:, j * TC:(j + 1) * TC],
                                         mub[:])
                    ybt = gtp.tile([128, TC], F16, tag="gt")
                    nc.vector.tensor_mul(ybt[:], ya[:], rsb[:])
                    yct = gtp.tile([128, TC], F16, tag="gt")
                    nc.scalar.activation(
                        yct[:], ybt[:], AF.Identity,
                        bias=cvec[:, LB + j:LB + j + 1],
                        scale=cvec[:, LW + j:LW + j + 1])
                    nc.vector.tensor_mul(sry[:, j * TC:(j + 1) * TC],
                                         yct[:], srb)

                pending_out.append((b, ch, sry))
                if last_chunk:
                    while pending_out:
                        emit_out(*pending_out.pop(0))

    nc.compile()
    return nc


def _pack(v):
    return np.ascontiguousarray(v.reshape(NCT, 128).T.astype(np.float32))


def kernel(x, Wk, Wv, Wr, Wo, ln_w, ln_b, spatial_decay, spatial_first,
           mix_k, mix_v, mix_r, H, W):
    x = np.asarray(x, dtype=np.float32)
    assert int(H) == 32 and int(W) == 32 and x.shape == (B, T, C)

    if "nc" not in _CACHE:
        _CACHE["nc"] = build()
    nc = _CACHE["nc"]

    w_eff = -np.exp(np.asarray(spatial_decay, np.float64) / T)
    u_eff = np.asarray(spatial_first, np.float64) / T
    cvec = np.concatenate([
        _pack(np.asarray(mix_k, np.float32) - 1.0),
        _pack(np.asarray(mix_v, np.float32) - 1.0),
        _pack(np.asarray(mix_r, np.float32) - 1.0),
        _pack(np.exp(w_eff).astype(np.float32)),
        _pack(np.exp(u_eff).astype(np.float32)),
        _pack(np.asarray(ln_w, np.float32)),
        _pack(np.asarray(ln_b, np.float32)),
    ], axis=1)
    def _jmajor(W):
        wT = np.asarray(W, np.float32).T.astype(np.float16)   # [C, D]
        return np.ascontiguousarray(
            wT.reshape(C, NCT, 128).transpose(1, 0, 2))       # [j, c, dd]
    wk16 = _jmajor(Wk)
    wv16 = _jmajor(Wv)
    # R weights: fp8, prescaled, packed [c_local(128), i2(4), slot(2), j(8), d(128)]
    wrT = np.asarray(Wr, np.float32).T * SR_W                 # [c, d]
    wr8 = wrT.reshape(4, 2, 128, NCT, 128).transpose(2, 0, 1, 3, 4)
    wr8 = np.ascontiguousarray(wr8.reshape(128, -1)).astype(E4)
    wo16 = np.ascontiguousarray(np.asarray(Wo, np.float32).T).astype(np.float16)
    x_t = np.ascontiguousarray(x.transpose(0, 2, 1)).astype(np.float16)

    in_maps = []
    for c in range(NCORES):
        in_maps.append({
            "x": x_t[c * BL:(c + 1) * BL],
            "wk": wk16, "wv": wv16, "wr8": wr8, "wo": wo16,
            "cvec": cvec,
        })
    last_err = None
    for _attempt in range(3):
        try:
            res = run_bass_kernel_spmd(nc, in_maps,
                                       core_ids=list(range(NCORES)))
            break
        except Exception as e:  # transient device wedge: retry
            last_err = e
            import time as _time
            _time.sleep(2.0)
    else:
        raise last_err
    out = np.concatenate([res.results[c]["out"] for c in range(NCORES)], axis=0)
    return out.astype(np.float32)


# revision 12
# speedup vs baseline: 1.0606x; 1.0526x over previous
"""Vision-RWKV (RWKV-v4 spatial mix) encoder block on 8 Trainium2 NeuronCores.

Strategy: data-parallel over batch B=16 -> 2 batches per core, no collectives.
Layout: channel-major [c, t] on-chip (x host-transposed). The WKV recurrence
P_t = e^w * P_{t-1} + e^{k_t} v_t runs as a hardware tensor_tensor_scan along
the free (token) dim. K/V/O matmuls run in fp16 (full PE rate); the R
projection runs in fp8e4 DoubleRow mode (2 contraction tiles per instruction
at 0.5 cyc/row) with weights host-prescaled by 256 and descaled for free via
the activation scale; the sigmoid gate is computed as 0.5*tanh(r/2)+0.5 so
Exp/Tanh/Square/Identity stay within one activation table set. e^u ~= 1
(|u|<=5e-4) is dropped. P-scan/num in fp16 (DVE 2x mode), q/den/recip fp32.
LayerNorm stats via PE ones-matmul (cross-partition reduction).

Self-contained: hardcodes B=16, T=1024, C=1024, H=W=32, 8 cores.
"""
import sys
sys.path.insert(0, "/opt/trn_rl_repo")

from contextlib import ExitStack

import numpy as np
import ml_dtypes

import concourse.bacc as bacc
import concourse.tile as tile
from concourse import mybir
from concourse.bass_utils import run_bass_kernel_spmd

dt = mybir.dt
AF = mybir.ActivationFunctionType
ALU = mybir.AluOpType
DRM = mybir.MatmulPerfMode.DoubleRow

B, T, C = 16, 1024, 1024
NCORES = 8
BL = B // NCORES          # batches per core
NCT = C // 128            # channel tiles (8)
TC = 512                  # token chunk
NCH = T // TC             # chunks per batch (2)
NTT = TC // 128           # token tiles per chunk (4)
EPS = 1e-5
SR_W = 256.0              # host prescale on R weights (fp8 subnormal avoidance)

F32, F16, F8 = dt.float32, dt.float16, dt.float8e4
E4 = ml_dtypes.float8_e4m3

# cvec column offsets (each vector packed as [128, 8], c_tile-major columns)
MK, MV, MR, EW, EU, LW, LB = 0, 8, 16, 24, 32, 40, 48
NVEC = 56

_CACHE = {}


def _emit_dshift(nc, d16, x16, i, c, eng=None):
    """d = x - qshift(x) for channel block i, chunk c, into d16[:, i*TC : (i+1)*TC].

    x16 is the full [128, 8*1024] fp16 plane (block i at cols i*1024).
    Within a block, col index = token t (0..1023). Chunk covers t in
    [c*TC, (c+1)*TC). Group g = i//2 determines the spatial shift.
    """
    g = i // 2
    xb = c * TC              # token base of this chunk
    xo = i * 1024 + xb       # col offset of this (block, chunk) in x16
    do = i * TC              # col offset in the chunk-local d16 plane
    last = c == NCH - 1
    first = c == 0

    engine = eng if eng is not None else nc.gpsimd

    def tt_sub(lo, hi, shift):
        # d[t] = x[t] - x[t - shift] over chunk-local t in [lo, hi)
        engine.tensor_tensor(
            d16[:, do + lo:do + hi],
            x16[:, xo + lo:xo + hi],
            x16[:, xo + lo - shift:xo + hi - shift],
            ALU.subtract)

    def fix(lo, hi):
        nc.vector.tensor_copy(d16[:, do + lo:do + hi], x16[:, xo + lo:xo + hi])

    def fix_strided(col):
        # copy x -> d at chunk-local t with t % 32 == col
        sr = x16[:, xo:xo + TC].rearrange("p (a q) -> p a q", q=32)
        dr = d16[:, do:do + TC].rearrange("p (a q) -> p a q", q=32)
        nc.vector.tensor_copy(dr[:, :, col:col + 1], sr[:, :, col:col + 1])

    if g == 0:        # xx = x(h, w-1), 0 at w==0  -> shift +1
        tt_sub(1 if first else 0, TC, 1)
        fix_strided(0)                                 # t % 32 == 0
    elif g == 1:      # xx = x(h, w+1), 0 at w==31 -> shift -1
        tt_sub(0, TC - 1 if last else TC, -1)
        fix_strided(31)                                # t % 32 == 31
    elif g == 2:      # xx = x(h-1, w), 0 at h==0  -> shift +32
        tt_sub(32 if first else 0, TC, 32)
        if first:
            fix(0, 32)
    else:             # xx = x(h+1, w), 0 at h==31 -> shift -32
        tt_sub(0, TC - 32 if last else TC, -32)
        if last:
            fix(TC - 32, TC)


def build():
    nc = bacc.Bacc("TRN2", target_bir_lowering=False, debug=False)

    x_d = nc.dram_tensor("x", [BL, C, T], F16, kind="ExternalInput")
    wk_d = nc.dram_tensor("wk", [NCT, C, 128], F16, kind="ExternalInput")
    wv_d = nc.dram_tensor("wv", [NCT, C, 128], F16, kind="ExternalInput")
    wr8_d = nc.dram_tensor("wr8", [128, 4 * 2 * NCT * 128], F8,
                           kind="ExternalInput")
    wo_d = nc.dram_tensor("wo", [C, C], F16, kind="ExternalInput")
    cvec_d = nc.dram_tensor("cvec", [128, NVEC], F32, kind="ExternalInput")
    out_d = nc.dram_tensor("out", [BL, T, C], F32, kind="ExternalOutput")

    with tile.TileContext(nc) as tc, ExitStack() as ctx:
        # ---- pools ----
        cons = ctx.enter_context(tc.tile_pool(name="cons", bufs=1))
        wp = ctx.enter_context(tc.tile_pool(name="wp", bufs=1))
        xpl = ctx.enter_context(tc.tile_pool(name="xpl", bufs=2))
        dpl = ctx.enter_context(tc.tile_pool(name="dpl", bufs=1))
        mxp = ctx.enter_context(tc.tile_pool(name="mxp", bufs=4))
        ekp = ctx.enter_context(tc.tile_pool(name="ekp", bufs=2))
        pqp = ctx.enter_context(tc.tile_pool(name="pqp", bufs=2))
        ndp = ctx.enter_context(tc.tile_pool(name="ndp", bufs=1))
        ypl = ctx.enter_context(tc.tile_pool(name="ypl", bufs=1))
        ysq = ctx.enter_context(tc.tile_pool(name="ysq", bufs=2))
        stp = ctx.enter_context(tc.tile_pool(name="stp", bufs=2))
        gtp = ctx.enter_context(tc.tile_pool(name="gtp", bufs=6))
        srp = ctx.enter_context(tc.tile_pool(name="srp", bufs=1))
        syp = ctx.enter_context(tc.tile_pool(name="syp", bufs=1))
        osg = ctx.enter_context(tc.tile_pool(name="osg", bufs=1))
        carp = ctx.enter_context(tc.tile_pool(name="carp", bufs=1))

        pp = ctx.enter_context(tc.tile_pool(name="pp", bufs=4, space="PSUM"))
        sps = ctx.enter_context(tc.tile_pool(name="sps", bufs=1, space="PSUM"))
        opl = ctx.enter_context(tc.tile_pool(name="opl", bufs=2, space="PSUM"))

        # ---- constants ----
        cvec = cons.tile([128, NVEC], F32)
        nc.sync.dma_start(cvec[:], cvec_d.ap()[:])
        ones16 = cons.tile([128, 1], F16)
        nc.vector.memset(ones16[:], 1.0 / C)
        eps_t = cons.tile([1, 1], F32)
        nc.vector.memset(eps_t[:], EPS)
        s2r = cons.tile([128, 1], F32)
        nc.vector.memset(s2r[:], 1.0 / (2.0 * SR_W))   # tanh(r/2) descale

        # PE warmup during the initial x/weight DMA wait: keeps the HAM
        # clock-gate and p-state warm so the first real matmuls run at 2.4GHz
        wu = cons.tile([128, 128], F16)
        nc.vector.memset(wu[:], 0.5)
        wu_ps = pp.tile([128, 128], F32, tag="proj")
        for _ in range(60):
            nc.tensor.matmul(wu_ps[:], wu[:], wu[:], start=True, stop=True)

        wk = wp.tile([128, NCT * C], F16, tag="wk")
        wv = wp.tile([128, NCT * C], F16, tag="wv")
        wr8 = wp.tile([128, 4, 2, NCT, 128], F8, tag="wr8")
        wo = wp.tile([128, NCT * C], F16, tag="wo")

        # ew16[:, j*TC:(j+1)*TC] = exp(w_eff) for channel block j, along free
        ew16 = cons.tile([128, NCT * TC], F16)
        nc.vector.memset(ew16[:], 1.0)
        for j in range(NCT):
            nc.vector.tensor_scalar(
                ew16[:, j * TC:(j + 1) * TC], ew16[:, j * TC:(j + 1) * TC],
                cvec[:, EW + j:EW + j + 1], None, ALU.mult)

        def dma_w(w_sb, w_d):
            nc.scalar.dma_start(
                w_sb[:].rearrange("p (i d) -> p i d", d=C),
                w_d.ap().rearrange("(i p) d -> p i d", p=128))

        def dma_x(b):
            x16 = xpl.tile([128, NCT * 1024], F16, tag="x16")
            half = NCT * 1024 // 2
            xsrc = x_d.ap()[b].rearrange("(i p) t -> p i t", p=128)
            for h in range(2):
                nc.sync.dma_start(
                    x16[:, h * half:(h + 1) * half]
                    .rearrange("p (i t) -> p i t", t=1024),
                    xsrc[:, h * 4:(h + 1) * 4, :])
            return x16

        pending_out = []

        def emit_out(ob, och, sry):
            for m in range(NTT):
                og = osg.tile([128, C], F32, tag="ostg")
                for dh in range(2):
                    o_ps = opl.tile([128, TC], F32, tag="oproj")
                    for i in range(NCT):
                        nc.tensor.matmul(
                            o_ps[:],
                            sry[:, i * TC + m * 128:i * TC + (m + 1) * 128],
                            wo[:, i * C + dh * TC:i * C + (dh + 1) * TC],
                            start=(i == 0), stop=(i == NCT - 1))
                    if ob == BL - 1 and och == NCH - 1:
                        nc.vector.tensor_copy(
                            og[:, dh * TC:(dh + 1) * TC], o_ps[:])
                        eng = nc.sync if dh == 0 else nc.scalar
                        trow = (och * NTT + m) * 128
                        eng.dma_start(
                            out_d.ap()[ob, trow:trow + 128,
                                       dh * TC:(dh + 1) * TC],
                            og[:, dh * TC:(dh + 1) * TC])
                    else:
                        nc.scalar.copy(og[:, dh * TC:(dh + 1) * TC], o_ps[:])
                if not (ob == BL - 1 and och == NCH - 1):
                    trow = (och * NTT + m) * 128
                    nc.sync.dma_start(
                        out_d.ap()[ob, trow:trow + 128, :], og[:])

        x16_first = dma_x(0)

        def dma_wj(eng, w_sb, w_d, j):
            eng.dma_start(
                w_sb[:, j * C:(j + 1) * C].rearrange("p (i d) -> p i d", d=128),
                w_d.ap()[j].rearrange("(i p) d -> p i d", p=128))
        for j in range(NCT):
            dma_wj(nc.scalar, wk, wk_d, j)
            dma_wj(nc.sync, wv, wv_d, j)

        for b in range(BL):
            # ---- ingest x (host-transposed, host-cast fp16): one DMA ----
            x16 = x16_first if b == 0 else _CACHE.pop("x16_next")

            if b == 0:
                nc.scalar.dma_start(
                    wr8[:].rearrange("p a b c d -> p (a b c d)"),
                    wr8_d.ap()[:])
                dma_w(wo, wo_d)

            carryP = carp.tile([128, NCT], F16, tag="cp")
            carryQ = carp.tile([128, NCT], F16, tag="cq")

            def emit_d(ch):
                d16 = dpl.tile([128, NCT * TC], F16, tag="d16")
                for i in range(NCT):
                    _emit_dshift(nc, d16, x16, i, ch, eng=nc.vector)
                return d16

            def emit_mix_block(ch, d16, mx, off, i):
                s = slice(i * TC, (i + 1) * TC)
                xs = slice(i * 1024 + ch * TC, i * 1024 + ch * TC + TC)
                nc.vector.scalar_tensor_tensor(
                    mx[:, s], d16[:, s], cvec[:, off + i:off + i + 1],
                    x16[:, xs], ALU.mult, ALU.add)

            def emit_mix(ch, d16, off, fp8=False):
                mx = mxp.tile([128, NCT * TC], F8 if fp8 else F16,
                              tag="mx8" if fp8 else "mx",
                              bufs=2 if fp8 else None)
                for i in range(NCT):
                    emit_mix_block(ch, d16, mx, off, i)
                return mx

            d_cur = emit_d(0)
            xr_head = mxp.tile([128, NCT * TC], F8, tag="mx8", bufs=2)
            mix_next = (emit_mix(0, d_cur, MK), emit_mix(0, d_cur, MV),
                        ("lazy", xr_head, d_cur))
            for ch in range(NCH):
                xk, xv, xr = mix_next
                xr_lazy = None
                if isinstance(xr, tuple):
                    xr_lazy, xr, d_lazy = xr
                y16 = ypl.tile([128, NCT * TC], F16, tag="y16")
                if ch + 1 < NCH:
                    d_nxt = emit_d(ch + 1)
                    xk_nxt = mxp.tile([128, NCT * TC], F16, tag="mx")

                for j in range(NCT):
                    # ---- k, v projections for output-channel block j ----
                    k_ps = pp.tile([128, TC], F32, tag="proj")
                    v_ps = pp.tile([128, TC], F32, tag="proj")
                    for w_sb, xm, ps in [(wk, xk, k_ps), (wv, xv, v_ps)]:
                        for i in range(NCT):
                            base = (j * NCT + i) * 128
                            nc.tensor.matmul(
                                ps[:],
                                w_sb[:, base:base + 128],
                                xm[:, i * TC:(i + 1) * TC],
                                start=(i == 0), stop=(i == NCT - 1))

                    ek = ekp.tile([128, TC], F16, tag="ek")
                    nc.scalar.activation(ek[:], k_ps[:], AF.Exp)
                    v16 = ekp.tile([128, TC], F16, tag="v16")
                    nc.scalar.copy(v16[:], v_ps[:])
                    ekv = ekp.tile([128, TC], F16, tag="ekv")
                    nc.vector.tensor_mul(ekv[:], ek[:], v16[:])

                    # ---- WKV scan (p,q fp16 2x-mode; e^u ~= 1 dropped) ----
                    pbuf = pqp.tile([128, TC + 1], F16, tag="pbuf")
                    qbuf = pqp.tile([128, TC + 1], F16, tag="qbuf")
                    if ch == 0:
                        nc.vector.memset(pbuf[:, 0:1], 0.0)
                        nc.vector.memset(qbuf[:, 0:1], 0.0)
                    else:
                        nc.scalar.copy(pbuf[:, 0:1], carryP[:, j:j + 1])
                        nc.scalar.copy(qbuf[:, 0:1], carryQ[:, j:j + 1])
                    ewj = ew16[:, j * TC:(j + 1) * TC]
                    nc.vector.tensor_tensor_scan(
                        pbuf[:, 1:TC + 1], ewj, ekv[:], pbuf[:, 0:1],
                        ALU.mult, ALU.add)
                    nc.vector.tensor_tensor_scan(
                        qbuf[:, 1:TC + 1], ewj, ek[:], qbuf[:, 0:1],
                        ALU.mult, ALU.add)
                    if ch != NCH - 1:
                        nc.scalar.copy(carryP[:, j:j + 1], pbuf[:, TC:TC + 1])
                        nc.scalar.copy(carryQ[:, j:j + 1], qbuf[:, TC:TC + 1])

                    # ---- y = (P_{t-1} + e^k v) / (Q_{t-1} + e^k) ----
                    num = ndp.tile([128, TC], F16, tag="num")
                    den = ndp.tile([128, TC], F32, tag="den")
                    nc.vector.tensor_add(num[:], pbuf[:, 0:TC], ekv[:])
                    nc.vector.tensor_add(den[:], qbuf[:, 0:TC], ek[:])
                    nc.vector.reciprocal_approx_fast(den[:], den[:])
                    yb = y16[:, j * TC:(j + 1) * TC]
                    nc.gpsimd.tensor_mul(yb, num[:], den[:])

                    if xr_lazy:
                        emit_mix_block(ch, d_lazy, xr, MR, j)
                    if ch + 1 < NCH:
                        emit_mix_block(ch + 1, d_nxt, xk_nxt, MK, j)

                while pending_out:
                    emit_out(*pending_out.pop(0))

                # ---- LN stats: lhsT ones = 1/C so psum rows are mu, E[y2]
                st_ps = sps.tile([1, 2 * TC], F32, tag="stat")
                for j in range(NCT):
                    yb = y16[:, j * TC:(j + 1) * TC]
                    ys = ysq.tile([128, TC], F16, tag="ysq")
                    nc.scalar.square(ys[:], yb)
                    nc.tensor.matmul(st_ps[:, 0:TC], ones16[:], yb,
                                     start=(j == 0), stop=(j == NCT - 1))
                    nc.tensor.matmul(st_ps[:, TC:2 * TC], ones16[:], ys[:],
                                     start=(j == 0), stop=(j == NCT - 1))

                # ---- r projection (fp8 DoubleRow) + tanh-gate ----
                xr8 = xr.rearrange("p (i t) -> p i t", t=TC)
                sr16 = srp.tile([128, NCT * TC], F16, tag="sr")
                for j in range(NCT):
                    r_ps = pp.tile([128, TC], F32, tag="proj")
                    for i2 in range(4):
                        nc.tensor.matmul(
                            r_ps[:],
                            wr8[:, i2, :, j, :],
                            xr8[:, 2 * i2:2 * i2 + 2, :],
                            start=(i2 == 0), stop=(i2 == 3),
                            perf_mode=DRM)
                    th = gtp.tile([128, TC], F16, tag="th", bufs=2)
                    nc.scalar.activation(th[:], r_ps[:], AF.Tanh, scale=s2r[:])
                    nc.vector.tensor_scalar(sr16[:, j * TC:(j + 1) * TC],
                                            th[:], 0.5, 0.5,
                                            ALU.mult, ALU.add)

                if ch == NCH - 1 and b + 1 < BL:
                    _CACHE["x16_next"] = dma_x(b + 1)

                # ---- next chunk's xv/xr fill DVE while PE runs r/out ----
                if ch + 1 < NCH:
                    mix_next = (xk_nxt, emit_mix(ch + 1, d_nxt, MV),
                                emit_mix(ch + 1, d_nxt, MR, fp8=True))

                # ---- LN stats post (rows on partition 0) ----
                mu16 = stp.tile([1, TC], F16, tag="mu16")
                nc.scalar.copy(mu16[:], st_ps[:, 0:TC])
                ms_t = stp.tile([1, TC], F32, tag="strow")
                nc.scalar.square(ms_t[:], st_ps[:, 0:TC])
                var_t = stp.tile([1, TC], F32, tag="strow")
                nc.vector.tensor_sub(var_t[:], st_ps[:, TC:2 * TC], ms_t[:])
                sd_t = stp.tile([1, TC], F32, tag="strow")
                nc.scalar.activation(sd_t[:], var_t[:], AF.Sqrt, bias=eps_t[:])
                nc.vector.reciprocal_approx_fast(sd_t[:], sd_t[:])
                rs16 = stp.tile([1, TC], F16, tag="rs16")
                nc.vector.tensor_scalar(rs16[:], sd_t[:], 1.0, None, ALU.mult)
                rsb = stp.tile([128, TC], F16, tag="bcast")
                nc.gpsimd.partition_broadcast(rsb[:], rs16[:])
                mub = stp.tile([128, TC], F16, tag="bcast")
                nc.gpsimd.partition_broadcast(mub[:], mu16[:])

                # ---- gate: sry = sr * ((y - mu)*rstd*lnw + lnb) ----
                sry = syp.tile([128, NCT * TC], F16, tag="sry")
                last_chunk = (b == BL - 1 and ch == NCH - 1)
                for j in range(NCT):
                    srb = sr16[:, j * TC:(j + 1) * TC]
                    ya = gtp.tile([128, TC], F16, tag="gt")
                    nc.gpsimd.tensor_sub(ya[:], y16[:, j * TC:(j + 1) * TC],
                                         mub[:])
                    ybt = gtp.tile([128, TC], F16, tag="gt")
                    nc.vector.tensor_mul(ybt[:], ya[:], rsb[:])
                    yct = gtp.tile([128, TC], F16, tag="gt")
                    nc.scalar.activation(
                        yct[:], ybt[:], AF.Identity,
                        bias=cvec[:, LB + j:LB + j + 1],
                        scale=cvec[:, LW + j:LW + j + 1])
                    nc.vector.tensor_mul(sry[:, j * TC:(j + 1) * TC],
                                         yct[:], srb)

                pending_out.append((b, ch, sry))
                if last_chunk:
                    while pending_out:
                        emit_out(*pending_out.pop(0))

    nc.compile()
    return nc


def _pack(v):
    return np.ascontiguousarray(v.reshape(NCT, 128).T.astype(np.float32))


def kernel(x, Wk, Wv, Wr, Wo, ln_w, ln_b, spatial_decay, spatial_first,
           mix_k, mix_v, mix_r, H, W):
    x = np.asarray(x, dtype=np.float32)
    assert int(H) == 32 and int(W) == 32 and x.shape == (B, T, C)

    if "nc" not in _CACHE:
        _CACHE["nc"] = build()
    nc = _CACHE["nc"]

    w_eff = -np.exp(np.asarray(spatial_decay, np.float64) / T)
    u_eff = np.asarray(spatial_first, np.float64) / T
    cvec = np.concatenate([
        _pack(np.asarray(mix_k, np.float32) - 1.0),
        _pack(np.asarray(mix_v, np.float32) - 1.0),
        _pack(np.asarray(mix_r, np.float32) - 1.0),
        _pack(np.exp(w_eff).astype(np.float32)),
        _pack(np.exp(u_eff).astype(np.float32)),
        _pack(np.asarray(ln_w, np.float32)),
        _pack(np.asarray(ln_b, np.float32)),
    ], axis=1)
    def _jmajor(W):
        wT = np.asarray(W, np.float32).T.astype(np.float16)   # [C, D]
        return np.ascontiguousarray(
            wT.reshape(C, NCT, 128).transpose(1, 0, 2))       # [j, c, dd]
    wk16 = _jmajor(Wk)
    wv16 = _jmajor(Wv)
    # R weights: fp8, prescaled, packed [c_local(128), i2(4), slot(2), j(8), d(128)]
    wrT = np.asarray(Wr, np.float32).T * SR_W                 # [c, d]
    wr8 = wrT.reshape(4, 2, 128, NCT, 128).transpose(2, 0, 1, 3, 4)
    wr8 = np.ascontiguousarray(wr8.reshape(128, -1)).astype(E4)
    wo16 = np.ascontiguousarray(np.asarray(Wo, np.float32).T).astype(np.float16)
    x_t = np.ascontiguousarray(x.transpose(0, 2, 1)).astype(np.float16)

    in_maps = []
    for c in range(NCORES):
        in_maps.append({
            "x": x_t[c * BL:(c + 1) * BL],
            "wk": wk16, "wv": wv16, "wr8": wr8, "wo": wo16,
            "cvec": cvec,
        })
    last_err = None
    for _attempt in range(3):
        try:
            res = run_bass_kernel_spmd(nc, in_maps,
                                       core_ids=list(range(NCORES)))
            break
        except Exception as e:  # transient device wedge: retry
            last_err = e
            import time as _time
            _time.sleep(2.0)
    else:
        raise last_err
    out = np.concatenate([res.results[c]["out"] for c in range(NCORES)], axis=0)
    return out.astype(np.float32)
